# revision 26
# baseline (speedup 1.0000x reference)
"""Trainium2 Bass kernel for nn_CorticalSheet (MoE routing / predictive-coder columns).

Strategy: expert-parallel over the C=16 columns across 8 NeuronCores (2 experts
per core). Each core computes, for its 2 experts, the full 5-matmul chain
(enc D->K, 3x K->K, dec K->D) over all B*T=4096 rows in a transposed
[feature, rows] layout (so no on-chip transposes are needed), plus all the
heavy reductions: residual sums over T, squared-error partials, z sums over T,
and entropy partial sums (S1 = sum_k exp(h), S2 = sum_k h*exp(h)) via
ones-vector matmuls for the cross-partition reduction.

The cheap O(B*D) glue (gate layernorm/top-k/softmax, gated combine, op/prior
heads, KL) runs on host in numpy.

Two device-kernel variants:
- "bf16" (default): fully fused single pass per expert, all weights resident
  in SBUF, no DRAM round-trips; entropy chunk sums tree-reduced on DVE so the
  PE only runs one ones-matmul per quantity per block. ~1.65-1.9 ms HW time
  (chip P-state dependent), worst rel err ~1.2e-3.
- "f32r": 3-pass structure with rolling 32KB weight slots and fp32r (TF32-like)
  matmuls. ~2.19 ms, worst rel err ~8e-5. Select with KERNEL_VARIANT env.
"""

import sys

sys.path.insert(0, "/opt/trn_rl_repo")

import numpy as np

import concourse.bass as bass
import concourse.mybir as mybir
import concourse.tile as tile
from concourse.bass_utils import run_bass_kernel_spmd


def _ensure_axon_ntff_hook():
    """Register the axon NTFF profiling hook if the container's antenv stub
    lacks axon_hooks (boot skips registration silently in that case)."""
    import types

    try:
        import antenv.axon_hooks  # noqa: F401
        return
    except ImportError:
        pass
    try:
        from trn_agent_boot.trn_boot import _ntff_profile_via_ctypes
        hook = _ntff_profile_via_ctypes("/opt/axon/libaxon_pjrt.so")
    except Exception:
        hook = None
    mod = types.ModuleType("antenv.axon_hooks")
    state = {"hook": hook}
    mod.get_axon_ntff_profile_hook = lambda: state["hook"]
    mod.set_axon_ntff_profile_hook = lambda h: state.update(hook=h)
    sys.modules["antenv.axon_hooks"] = mod
    import antenv
    antenv.axon_hooks = mod


_ensure_axon_ntff_hook()

# ---- problem constants (hardcoded per contract) ----
B, T, D = 256, 16, 2048
C = 16
K = 1024
DEPTH = 4
TOPK = 4
TEMP = 1.0
SMAX = 10.0

P = 128
DO = D // P          # 16 d-chunks
KO = K // P          # 8 k-chunks
N = B * T            # 4096 rows
NCORES = 8
NE = C // NCORES     # 2 experts per core

NBA = 512            # rows per block, enc pass
NBB = 512            # rows per block, hidden pass
NBC = 512            # rows per block, dec pass

f32 = mybir.dt.float32
f32r = mybir.dt.float32r
bf16 = mybir.dt.bfloat16
AF = mybir.ActivationFunctionType
ALU = mybir.AluOpType
AXL = mybir.AxisListType
GELU_FN = AF.Gelu_apprx_tanh  # swapped to AF.Tanh for CoreSim (no Gelu LUT in sim)

_CACHE = {}
VARIANT = "bf16"  # default device-kernel variant ("f32r" fallback: ~2.19ms, rel err 8e-5)

LAST_RESULTS = None  # BassKernelResults of the most recent device run


def _legalize_multi_waits(nc):
    """Split instructions carrying >1 semaphore wait (or >1 update).

    The TPB EVENTS field holds exactly one wait and one update; this
    walrus_driver refuses to synthesize extra wait uops ("Too many sync wait
    commands"), so hoist the surplus waits into standalone EventSemaphore
    instructions on the same engine immediately before the instruction (and
    surplus updates into one immediately after).
    """
    n_ev = 0
    for func in nc.m.functions:
        for bb in func.blocks:
            insts = bb.instructions
            out = []
            changed = False
            for inst in insts:
                si = inst.sync_info
                waits = list(si.on_wait) if si is not None and si.on_wait else []
                updates = list(si.on_update) if si is not None and si.on_update else []
                if len(waits) > 1:
                    for w in waits[:-1]:
                        n_ev += 1
                        ev = mybir.InstEventSemaphore(
                            name=f"EVW-{n_ev}", ins=[], outs=[],
                            sync_info=mybir.SyncInfo(on_wait=[w], on_update=[]),
                        )
                        ev.engine = inst.engine
                        out.append(ev)
                    si.on_wait = [waits[-1]]
                    changed = True
                out.append(inst)
                if len(updates) > 1:
                    for u in updates[1:]:
                        n_ev += 1
                        ev = mybir.InstEventSemaphore(
                            name=f"EVU-{n_ev}", ins=[], outs=[],
                            sync_info=mybir.SyncInfo(on_wait=[], on_update=[u]),
                        )
                        ev.engine = inst.engine
                        out.append(ev)
                    si.on_update = [updates[0]]
                    changed = True
            if changed:
                bb.instructions = out
    return n_ev


def _build_bass():
    """Build the per-core Bass program (same program all 8 cores; data differs)."""
    nc = bass.Bass(trn_type="TRN2")

    xT = nc.dram_tensor("xT", [P, DO, N], f32r, kind="ExternalInput")
    w0 = [nc.dram_tensor(f"w0_{e}", [P, DO, K], f32r, kind="ExternalInput") for e in range(NE)]
    w1 = [nc.dram_tensor(f"w1_{e}", [P, KO, K], f32r, kind="ExternalInput") for e in range(NE)]
    w2 = [nc.dram_tensor(f"w2_{e}", [P, KO, K], f32r, kind="ExternalInput") for e in range(NE)]
    w3 = [nc.dram_tensor(f"w3_{e}", [P, KO, K], f32r, kind="ExternalInput") for e in range(NE)]
    wd = [nc.dram_tensor(f"wd_{e}", [P, KO, D], f32r, kind="ExternalInput") for e in range(NE)]
    b0 = [nc.dram_tensor(f"b0_{e}", [P, KO], f32, kind="ExternalInput") for e in range(NE)]
    b1 = [nc.dram_tensor(f"b1_{e}", [P, KO], f32, kind="ExternalInput") for e in range(NE)]
    b2 = [nc.dram_tensor(f"b2_{e}", [P, KO], f32, kind="ExternalInput") for e in range(NE)]
    b3 = [nc.dram_tensor(f"b3_{e}", [P, KO], f32, kind="ExternalInput") for e in range(NE)]
    bd = [nc.dram_tensor(f"bd_{e}", [P, DO], f32, kind="ExternalInput") for e in range(NE)]

    res_out = nc.dram_tensor("res_out", [NE, P, DO, B], f32, kind="ExternalOutput")
    z_out = nc.dram_tensor("z_out", [NE, P, KO, B], f32, kind="ExternalOutput")
    sq_out = nc.dram_tensor("sq_out", [NE, P, DO], f32, kind="ExternalOutput")
    ent_out = nc.dram_tensor("ent_out", [NE, 2, N], f32, kind="ExternalOutput")

    HDO = DO // 2  # 8
    HKO = KO // 2  # 4
    with tile.TileContext(nc) as tc:
        with (
            tc.tile_pool(name="dram", bufs=1, space="DRAM") as dpool,
            tc.tile_pool(name="const", bufs=1) as cpool,
            tc.tile_pool(name="wg", bufs=3) as wpool,
            tc.tile_pool(name="small", bufs=1) as smpool,
            tc.tile_pool(name="psg", bufs=6, space="PSUM") as psg,
            tc.tile_pool(name="ps1g", bufs=1, space="PSUM") as ps1g,
        ):
            ones_f = cpool.tile([P, 1], f32)
            nc.vector.memset(ones_f[:], 1.0)
            ones = ones_f[:].bitcast(f32r)
            # all biases, both experts, loaded up front (tiny)
            bias = {}
            for e in range(NE):
                for nm, src, w in (("b0", b0[e], KO), ("b1", b1[e], KO),
                                   ("b2", b2[e], KO), ("b3", b3[e], KO),
                                   ("bd", bd[e], DO)):
                    t = cpool.tile([P, w], f32, tag=f"{nm}_{e}")
                    nc.gpsimd.dma_start(t[:], src[:])
                    bias[(nm, e)] = t

            for e in range(NE):
                h0d = dpool.tile([P, KO, N], f32r, tag="h0")
                h3d = dpool.tile([P, KO, N], f32r, tag="h3")

                # ---------- pass A: enc (X -> h0) ----------
                # weights split by output column into two 32KB rolling slots
                w0a = wpool.tile([P, DO, K // 2], f32r, tag="w")
                nc.sync.dma_start(w0a[:], w0[e][:, :, :K // 2])
                w0b = wpool.tile([P, DO, K // 2], f32r, tag="w")
                nc.sync.dma_start(w0b[:], w0[e][:, :, K // 2:])
                b0t = bias[("b0", e)]
                with tc.tile_pool(name=f"aA{e}", bufs=3) as apool:
                    for blk in range(N // NBA):
                        sl = slice(blk * NBA, (blk + 1) * NBA)
                        x0 = apool.tile([P, HDO, NBA], f32r, tag="x")
                        nc.sync.dma_start(x0[:], xT[:, :HDO, sl])
                        x1 = apool.tile([P, HDO, NBA], f32r, tag="x")
                        nc.sync.dma_start(x1[:], xT[:, HDO:, sl])
                        h0t = apool.tile([P, KO, NBA], f32r, tag="h")
                        for ko in range(KO):
                            wt = w0a if ko < HKO else w0b
                            koo = ko % HKO
                            ps = psg.tile([P, NBA], f32, tag="mm")
                            for kd in range(DO):
                                xx = x0 if kd < HDO else x1
                                nc.tensor.matmul(
                                    ps[:],
                                    wt[:, kd, koo * P:(koo + 1) * P],
                                    xx[:, kd % HDO, :],
                                    start=(kd == 0),
                                    stop=(kd == DO - 1),
                                )
                            nc.scalar.activation(
                                h0t[:, ko, :], ps[:], GELU_FN,
                                bias=b0t[:, ko:ko + 1],
                            )
                        nc.sync.dma_start(h0d[:, :, sl], h0t[:])

                # ---------- pass B: hidden x3 (h0 -> h3) ----------
                wts = []
                for wi in (w1[e], w2[e], w3[e]):
                    wt = wpool.tile([P, KO, K], f32r, tag="w")
                    nc.sync.dma_start(wt[:], wi[:])
                    wts.append(wt)
                bts = [bias[("b1", e)], bias[("b2", e)], bias[("b3", e)]]
                with (
                    tc.tile_pool(name=f"aB{e}", bufs=2) as apool,
                    tc.tile_pool(name=f"sB{e}", bufs=1) as spool,
                ):
                    for blk in range(N // NBB):
                        sl = slice(blk * NBB, (blk + 1) * NBB)
                        hin = apool.tile([P, KO, NBB], f32r, tag="hin")
                        nc.sync.dma_start(hin[:], h0d[:, :, sl])
                        h1t = spool.tile([P, KO, NBB], f32r, tag="h1")
                        h2t = spool.tile([P, KO, NBB], f32r, tag="h2")
                        h3t = apool.tile([P, KO, NBB], f32r, tag="h3o")
                        for src, dst, li in ((hin, h1t, 0), (h1t, h2t, 1), (h2t, h3t, 2)):
                            for ko in range(KO):
                                ps = psg.tile([P, NBB], f32, tag="mm")
                                for kd in range(KO):
                                    nc.tensor.matmul(
                                        ps[:],
                                        wts[li][:, kd, ko * P:(ko + 1) * P],
                                        src[:, kd, :],
                                        start=(kd == 0),
                                        stop=(kd == KO - 1),
                                    )
                                nc.scalar.activation(
                                    dst[:, ko, :], ps[:], GELU_FN,
                                    bias=bts[li][:, ko:ko + 1],
                                )
                        nc.sync.dma_start(h3d[:, :, sl], h3t[:])

                # ---------- pass C: dec + stats ----------
                wda = wpool.tile([P, KO, D // 2], f32r, tag="w")
                nc.sync.dma_start(wda[:], wd[e][:, :, :D // 2])
                wdb = wpool.tile([P, KO, D // 2], f32r, tag="w")
                nc.sync.dma_start(wdb[:], wd[e][:, :, D // 2:])
                bdt = bias[("bd", e)]

                sq_sb = smpool.tile([P, DO], f32, tag=f"sq{e}")
                nc.vector.memset(sq_sb[:], 0.0)

                with (
                    tc.tile_pool(name=f"aC{e}", bufs=2) as apool,
                    tc.tile_pool(name=f"eC{e}", bufs=2) as epool,
                    tc.tile_pool(name=f"e1C{e}", bufs=1) as e1pool,
                ):
                    bblk = NBC // T  # batch entries per block
                    for blk in range(N // NBC):
                        sl = slice(blk * NBC, (blk + 1) * NBC)
                        bsl = slice(blk * bblk, (blk + 1) * bblk)
                        h3t = apool.tile([P, KO, NBC], f32r, tag="h3in")
                        nc.sync.dma_start(h3t[:], h3d[:, :, sl])
                        xc0 = apool.tile([P, HDO, NBC], f32r, tag="xc")
                        nc.sync.dma_start(xc0[:], xT[:, :HDO, sl])
                        xc1 = apool.tile([P, HDO, NBC], f32r, tag="xc")
                        nc.sync.dma_start(xc1[:], xT[:, HDO:, sl])

                        # entropy partials + z sums
                        ps1 = ps1g.tile([1, NBC], f32, tag="s1")
                        ps2 = ps1g.tile([1, NBC], f32, tag="s2")
                        z_blk = epool.tile([P, KO, bblk], f32, tag="zblk")
                        for kc in range(KO):
                            et = epool.tile([P, NBC], f32r, tag="e")
                            nc.scalar.activation(et[:], h3t[:, kc, :], AF.Exp)
                            het = epool.tile([P, NBC], f32r, tag="he")
                            nc.vector.tensor_tensor(het[:], et[:], h3t[:, kc, :], ALU.mult)
                            nc.tensor.matmul(ps1[:], ones[:, :1], et[:],
                                             start=(kc == 0), stop=(kc == KO - 1))
                            nc.tensor.matmul(ps2[:], ones[:, :1], het[:],
                                             start=(kc == 0), stop=(kc == KO - 1))
                            nc.vector.tensor_reduce(
                                z_blk[:, kc, :],
                                h3t[:, kc, :].rearrange("p (b t) -> p b t", t=T),
                                axis=AXL.X, op=ALU.add,
                            )
                        nc.sync.dma_start(z_out[e][:, :, bsl], z_blk[:])
                        s1t = e1pool.tile([1, NBC], f32, tag="s1sb")
                        nc.vector.tensor_copy(s1t[:], ps1[:])
                        nc.sync.dma_start(ent_out[e, 0, sl], s1t[:])
                        s2t = e1pool.tile([1, NBC], f32, tag="s2sb")
                        nc.vector.tensor_copy(s2t[:], ps2[:])
                        nc.sync.dma_start(ent_out[e, 1, sl], s2t[:])

                        # dec matmuls + error stats
                        res_blk = epool.tile([P, DO, bblk], f32, tag="resblk")
                        for dc in range(DO):
                            wt = wda if dc < HDO else wdb
                            dco = dc % HDO
                            ps = psg.tile([P, NBC], f32, tag="mm")
                            for kd in range(KO):
                                nc.tensor.matmul(
                                    ps[:],
                                    wt[:, kd, dco * P:(dco + 1) * P],
                                    h3t[:, kd, :],
                                    start=(kd == 0),
                                    stop=(kd == KO - 1),
                                )
                            err0 = epool.tile([P, NBC], f32, tag="err0")
                            nc.scalar.activation(err0[:], ps[:], AF.Identity,
                                                 bias=bdt[:, dc:dc + 1])
                            err = epool.tile([P, NBC], f32, tag="err")
                            xx = xc0 if dc < HDO else xc1
                            nc.vector.tensor_tensor(err[:], err0[:], xx[:, dc % HDO, :], ALU.subtract)
                            scr = e1pool.tile([P, NBC], f32, tag="scr")
                            sqp = e1pool.tile([P, 1], f32, tag="sqp")
                            nc.vector.tensor_tensor(scr[:], err[:], err[:], ALU.mult)
                            nc.vector.tensor_reduce(
                                sqp[:], scr[:], axis=AXL.X, op=ALU.add)
                            nc.vector.tensor_tensor(
                                sq_sb[:, dc:dc + 1], sq_sb[:, dc:dc + 1], sqp[:], ALU.add)
                            nc.vector.tensor_reduce(
                                res_blk[:, dc, :],
                                err.rearrange("p (b t) -> p b t", t=T),
                                axis=AXL.X, op=ALU.add,
                            )
                        nc.sync.dma_start(res_out[e][:, :, bsl], res_blk[:])

                nc.sync.dma_start(sq_out[e], sq_sb[:])

    _legalize_multi_waits(nc)
    return nc


def _build_bass_bf16():
    """Fully-fused bf16 variant: all weights resident per expert, the whole
    enc->h1->h2->h3->dec chain runs per row-block with no DRAM round-trips."""
    nc = bass.Bass(trn_type="TRN2")

    xT = nc.dram_tensor("xT", [P, DO, N], bf16, kind="ExternalInput")
    w0 = [nc.dram_tensor(f"w0_{e}", [P, DO, K], bf16, kind="ExternalInput") for e in range(NE)]
    w1 = [nc.dram_tensor(f"w1_{e}", [P, KO, K], bf16, kind="ExternalInput") for e in range(NE)]
    w2 = [nc.dram_tensor(f"w2_{e}", [P, KO, K], bf16, kind="ExternalInput") for e in range(NE)]
    w3 = [nc.dram_tensor(f"w3_{e}", [P, KO, K], bf16, kind="ExternalInput") for e in range(NE)]
    wd = [nc.dram_tensor(f"wd_{e}", [P, KO, D], bf16, kind="ExternalInput") for e in range(NE)]
    b0 = [nc.dram_tensor(f"b0_{e}", [P, KO], f32, kind="ExternalInput") for e in range(NE)]
    b1 = [nc.dram_tensor(f"b1_{e}", [P, KO], f32, kind="ExternalInput") for e in range(NE)]
    b2 = [nc.dram_tensor(f"b2_{e}", [P, KO], f32, kind="ExternalInput") for e in range(NE)]
    b3 = [nc.dram_tensor(f"b3_{e}", [P, KO], f32, kind="ExternalInput") for e in range(NE)]
    bd = [nc.dram_tensor(f"bd_{e}", [P, DO], f32, kind="ExternalInput") for e in range(NE)]

    res_out = nc.dram_tensor("res_out", [NE, P, DO, B], f32, kind="ExternalOutput")
    z_out = nc.dram_tensor("z_out", [NE, P, KO, B], f32, kind="ExternalOutput")
    sq_out = nc.dram_tensor("sq_out", [NE, P, DO], f32, kind="ExternalOutput")
    ent_out = nc.dram_tensor("ent_out", [NE, 2, N], f32, kind="ExternalOutput")

    NB = 512
    bblk = NB // T
    with tile.TileContext(nc) as tc:
        with (
            tc.tile_pool(name="const", bufs=1) as cpool,
            tc.tile_pool(name="wg", bufs=1) as wpool,
            tc.tile_pool(name="ap", bufs=2) as apool,
            tc.tile_pool(name="hp", bufs=2) as hpool,
            tc.tile_pool(name="ep", bufs=2) as epool,
            tc.tile_pool(name="e1", bufs=1) as e1pool,
            tc.tile_pool(name="small", bufs=1) as smpool,
            tc.tile_pool(name="psg", bufs=6, space="PSUM") as psg,
            tc.tile_pool(name="ps1g", bufs=1, space="PSUM") as ps1g,
        ):
            ones_f = cpool.tile([P, 1], f32)
            nc.vector.memset(ones_f[:], 1.0)
            ones = cpool.tile([P, 1], bf16)
            nc.vector.tensor_copy(ones[:], ones_f[:])
            bias = {}
            for e in range(NE):
                for nm, src, w in (("b0", b0[e], KO), ("b1", b1[e], KO),
                                   ("b2", b2[e], KO), ("b3", b3[e], KO),
                                   ("bd", bd[e], DO)):
                    t = cpool.tile([P, w], f32, tag=f"{nm}_{e}")
                    nc.gpsimd.dma_start(t[:], src[:])
                    bias[(nm, e)] = t

            for e in range(NE):
                w0t = wpool.tile([P, DO, K], bf16, tag="w0")
                nc.sync.dma_start(w0t[:], w0[e][:])
                # first x block right behind w0 so the first matmuls start early
                xt0 = apool.tile([P, DO, NB], bf16, tag="x")
                nc.sync.dma_start(xt0[:], xT[:, :, 0:NB])
                wts = []
                for li, wi in enumerate((w1[e], w2[e], w3[e])):
                    wt = wpool.tile([P, KO, K], bf16, tag=f"w{li + 1}")
                    nc.sync.dma_start(wt[:], wi[:])
                    wts.append(wt)
                wdt = wpool.tile([P, KO, D], bf16, tag="wd")
                nc.sync.dma_start(wdt[:], wd[e][:])

                sq_sb = smpool.tile([P, DO], f32, tag=f"sq{e}")
                nc.vector.memset(sq_sb[:], 0.0)

                NBLK = N // NB
                xts = {0: xt0}
                h0ts = {}

                def enc_slice(blk, ko_lo, ko_hi, e=e, w0t=w0t):
                    # a few output chunks of next block's enc, spliced into the
                    # current block's chain to fill PE layer-transition bubbles
                    for ko in range(ko_lo, ko_hi):
                        ps = psg.tile([P, NB], f32, tag="mm")
                        for kd in range(DO):
                            nc.tensor.matmul(
                                ps[:], w0t[:, kd, ko * P:(ko + 1) * P],
                                xts[blk][:, kd, :],
                                start=(kd == 0), stop=(kd == DO - 1))
                        nc.scalar.activation(h0ts[blk][:, ko, :], ps[:], GELU_FN,
                                             bias=bias[("b0", e)][:, ko:ko + 1])

                h0ts[0] = hpool.tile([P, KO, NB], bf16, tag="ha", name="h0t")
                enc_slice(0, 0, KO)
                # splice points after h1/h2/h3 of the current block
                slices = ((0, 3), (3, 6), (6, KO))
                for blk in range(NBLK):
                    sl = slice(blk * NB, (blk + 1) * NB)
                    bsl = slice(blk * bblk, (blk + 1) * bblk)
                    xt = xts[blk]
                    nxt = blk + 1
                    if nxt < NBLK:
                        xnt = apool.tile([P, DO, NB], bf16, tag="x")
                        nc.sync.dma_start(xnt[:], xT[:, :, nxt * NB:(nxt + 1) * NB])
                        xts[nxt] = xnt
                        h0ts[nxt] = hpool.tile([P, KO, NB], bf16, tag="ha", name="h0t")

                    # hidden x3 (ping-pong ha/hb), enc(next) spliced after each
                    hcur = h0ts.pop(blk)
                    for li in range(3):
                        hnxt = hpool.tile([P, KO, NB], bf16,
                                          tag="hb" if li % 2 == 0 else "ha")
                        for ko in range(KO):
                            ps = psg.tile([P, NB], f32, tag="mm")
                            for kd in range(KO):
                                nc.tensor.matmul(
                                    ps[:], wts[li][:, kd, ko * P:(ko + 1) * P], hcur[:, kd, :],
                                    start=(kd == 0), stop=(kd == KO - 1))
                            nc.scalar.activation(hnxt[:, ko, :], ps[:], GELU_FN,
                                                 bias=bias[(f"b{li + 1}", e)][:, ko:ko + 1])
                        hcur = hnxt
                        if nxt < NBLK:
                            enc_slice(nxt, *slices[li])
                    h3t = hcur

                    # entropy partials + z sums
                    ps1 = ps1g.tile([1, NB], f32, tag="s1")
                    ps2 = ps1g.tile([1, NB], f32, tag="s2")
                    z_blk = epool.tile([P, KO, bblk], f32, tag="zblk")
                    esum = epool.tile([P, NB], bf16, tag="esum")
                    hesum = epool.tile([P, NB], bf16, tag="hesum")
                    for kc in range(KO):
                        if kc == 0:
                            nc.scalar.activation(esum[:], h3t[:, kc, :], AF.Exp)
                            nc.vector.tensor_tensor(hesum[:], esum[:], h3t[:, kc, :], ALU.mult)
                        else:
                            et = epool.tile([P, NB], bf16, tag="e")
                            nc.scalar.activation(et[:], h3t[:, kc, :], AF.Exp)
                            het = epool.tile([P, NB], bf16, tag="he")
                            nc.vector.tensor_tensor(het[:], et[:], h3t[:, kc, :], ALU.mult)
                            nc.vector.tensor_tensor(esum[:], esum[:], et[:], ALU.add)
                            nc.vector.tensor_tensor(hesum[:], hesum[:], het[:], ALU.add)
                        nc.vector.tensor_reduce(
                            z_blk[:, kc, :],
                            h3t[:, kc, :].rearrange("p (b t) -> p b t", t=T),
                            axis=AXL.X, op=ALU.add)
                    nc.sync.dma_start(z_out[e][:, :, bsl], z_blk[:])

                    # dec + error stats
                    res_blk = epool.tile([P, DO, bblk], f32, tag="resblk")
                    for dc in range(DO):
                        ps = psg.tile([P, NB], f32, tag="mm")
                        for kd in range(KO):
                            nc.tensor.matmul(
                                ps[:], wdt[:, kd, dc * P:(dc + 1) * P], h3t[:, kd, :],
                                start=(kd == 0), stop=(kd == KO - 1))
                        err0 = epool.tile([P, NB], f32, tag="err0")
                        nc.scalar.activation(err0[:], ps[:], AF.Identity,
                                             bias=bias[("bd", e)][:, dc:dc + 1])
                        xf = e1pool.tile([P, NB], f32, tag="xf")
                        nc.scalar.copy(xf[:], xt[:, dc, :])
                        err = epool.tile([P, NB], f32, tag="err")
                        nc.vector.tensor_tensor(err[:], err0[:], xf[:], ALU.subtract)
                        scr = e1pool.tile([P, NB], f32, tag="scr")
                        sqp = e1pool.tile([P, 1], f32, tag="sqp")
                        nc.vector.tensor_tensor(scr[:], err[:], err[:], ALU.mult)
                        nc.vector.tensor_reduce(sqp[:], scr[:], axis=AXL.X, op=ALU.add)
                        nc.vector.tensor_tensor(
                            sq_sb[:, dc:dc + 1], sq_sb[:, dc:dc + 1], sqp[:], ALU.add)
                        nc.vector.tensor_reduce(
                            res_blk[:, dc, :],
                            err.rearrange("p (b t) -> p b t", t=T),
                            axis=AXL.X, op=ALU.add)
                    nc.sync.dma_start(res_out[e][:, :, bsl], res_blk[:])
                    # entropy cross-partition matmuls last: esum/hesum long done
                    nc.tensor.matmul(ps1[:], ones[:, :1], esum[:], start=True, stop=True)
                    nc.tensor.matmul(ps2[:], ones[:, :1], hesum[:], start=True, stop=True)
                    s1t = e1pool.tile([1, NB], f32, tag="s1sb")
                    nc.vector.tensor_copy(s1t[:], ps1[:])
                    nc.sync.dma_start(ent_out[e, 0, sl], s1t[:])
                    s2t = e1pool.tile([1, NB], f32, tag="s2sb")
                    nc.vector.tensor_copy(s2t[:], ps2[:])
                    nc.sync.dma_start(ent_out[e, 1, sl], s2t[:])

                nc.sync.dma_start(sq_out[e], sq_sb[:])

    _legalize_multi_waits(nc)
    return nc


def _prep_inputs(inputs, wdtype=np.float32):
    """Host-side shard/layout prep -> list of 8 per-core input dicts."""
    tokens = np.ascontiguousarray(inputs["tokens"], dtype=np.float32)
    X = tokens.reshape(N, D)
    # xT[p, o, n] = X[n, o*128 + p]
    xT = np.ascontiguousarray(X.T.reshape(DO, P, N).transpose(1, 0, 2).astype(wdtype))

    ew0 = np.asarray(inputs["ew0"], dtype=np.float32)
    ew = np.asarray(inputs["ew"], dtype=np.float32)
    dw = np.asarray(inputs["dw"], dtype=np.float32)
    eb0 = np.asarray(inputs["eb0"], dtype=np.float32)
    eb = np.asarray(inputs["eb"], dtype=np.float32)
    db = np.asarray(inputs["db"], dtype=np.float32)

    in_maps = []
    for core in range(NCORES):
        m = {"xT": xT}
        for e in range(NE):
            c = core * NE + e
            m[f"w0_{e}"] = np.ascontiguousarray(
                ew0[c].reshape(DO, P, K).transpose(1, 0, 2).astype(wdtype))
            for i in range(DEPTH - 1):
                m[f"w{i + 1}_{e}"] = np.ascontiguousarray(
                    ew[i, c].reshape(KO, P, K).transpose(1, 0, 2).astype(wdtype))
                m[f"b{i + 1}_{e}"] = np.ascontiguousarray(
                    eb[i, c].reshape(KO, P).T)
            m[f"wd_{e}"] = np.ascontiguousarray(
                dw[c].reshape(KO, P, D).transpose(1, 0, 2).astype(wdtype))
            m[f"b0_{e}"] = np.ascontiguousarray(eb0[c].reshape(KO, P).T)
            m[f"bd_{e}"] = np.ascontiguousarray(db[c].reshape(DO, P).T)
        in_maps.append(m)
    return in_maps


def _gelu_tanh(x):
    return 0.5 * x * (1.0 + np.tanh(np.sqrt(2.0 / np.pi) * (x + 0.044715 * x ** 3)))


def _ln_np(x, s, b):
    m = x.mean(-1, keepdims=True)
    v = x.var(-1, keepdims=True)
    return (x - m) / np.sqrt(v + 1e-5) * s + b


def _softmax_np(x):
    x = x - x.max(-1, keepdims=True)
    ex = np.exp(x)
    return ex / ex.sum(-1, keepdims=True)


def kernel(**inputs):
    global LAST_RESULTS
    import os
    variant = os.environ.get("KERNEL_VARIANT", VARIANT)
    if variant not in _CACHE:
        _CACHE[variant] = _build_bass() if variant == "f32r" else _build_bass_bf16()
    nc = _CACHE[variant]

    import ml_dtypes
    wdtype = np.float32 if variant == "f32r" else ml_dtypes.bfloat16
    in_maps = _prep_inputs(inputs, wdtype)
    import os
    trace = bool(int(os.environ.get("KERNEL_TRACE", "0")))
    results = run_bass_kernel_spmd(
        nc, in_maps, core_ids=list(range(NCORES)), trace=trace,
    )
    LAST_RESULTS = results
    per_core = results.results

    # ---- reassemble device outputs ----
    residuals = np.empty((C, B, D), np.float64)
    zc = np.empty((C, B, K), np.float64)
    sq_total = 0.0
    ent_sum = 0.0
    for core in range(NCORES):
        r = per_core[core]
        for e in range(NE):
            c = core * NE + e
            # res_out[e]: [P, DO, B] -> [B, DO*P] with d = o*128+p
            residuals[c] = r["res_out"][e].transpose(2, 1, 0).reshape(B, D) / T
            zc[c] = r["z_out"][e].transpose(2, 1, 0).reshape(B, K) / T
            sq_total += float(r["sq_out"][e].astype(np.float64).sum())
            s1 = r["ent_out"][e, 0].astype(np.float64)
            s2 = r["ent_out"][e, 1].astype(np.float64)
            ent_sum += float((np.log(s1) - s2 / s1).sum())

    pc_recon = np.float32(sq_total / (C * B * T * D))
    pc_entropy = np.float32(ent_sum / (C * N))

    # ---- gate (host) ----
    tokens = np.asarray(inputs["tokens"], dtype=np.float32)
    gln_s = np.asarray(inputs["gln_s"], np.float32)
    gln_b = np.asarray(inputs["gln_b"], np.float32)
    gw = np.asarray(inputs["gw"], np.float32)
    gb = np.asarray(inputs["gb"], np.float32)

    x = tokens.mean(axis=1).astype(np.float64)
    logits = (_ln_np(x, gln_s, gln_b) @ gw + gb) / max(1e-4, TEMP)
    top_idx = np.argsort(-logits, axis=-1)[:, :TOPK]
    mask = np.zeros_like(logits)
    np.put_along_axis(mask, top_idx, 1.0, axis=-1)
    gp = _softmax_np(logits) * mask
    gate_probs = gp / np.maximum(gp.sum(-1, keepdims=True), 1e-6)

    # ---- gated combine ----
    residual = np.einsum("bc,cbd->bd", gate_probs, residuals)
    latent = np.einsum("bc,cbk->bk", gate_probs, zc)

    # ---- heads ----
    oln_s = np.asarray(inputs["oln_s"], np.float32)
    oln_b = np.asarray(inputs["oln_b"], np.float32)
    ow1 = np.asarray(inputs["ow1"], np.float32)
    ob1 = np.asarray(inputs["ob1"], np.float32)
    ow2 = np.asarray(inputs["ow2"], np.float32)
    ob2 = np.asarray(inputs["ob2"], np.float32)
    pln_s = np.asarray(inputs["pln_s"], np.float32)
    pln_b = np.asarray(inputs["pln_b"], np.float32)
    pw1 = np.asarray(inputs["pw1"], np.float32)
    pb1 = np.asarray(inputs["pb1"], np.float32)
    pw2 = np.asarray(inputs["pw2"], np.float32)
    pb2 = np.asarray(inputs["pb2"], np.float32)

    op_logits = _gelu_tanh(_ln_np(latent, oln_s, oln_b) @ ow1 + ob1) @ ow2 + ob2
    op_probs = _softmax_np(op_logits)
    prior_raw = 1.0 / (1.0 + np.exp(-(_gelu_tanh(_ln_np(latent, pln_s, pln_b) @ pw1 + pb1) @ pw2 + pb2)))
    smin = 1.0 / SMAX
    prior_scales = smin + (SMAX - smin) * prior_raw

    # ---- kl sparsity ----
    t = 0.05
    rho = np.clip((1.0 / (1.0 + np.exp(-latent))).mean(axis=0), 1e-6, 1.0 - 1e-6)
    kl = np.float32(
        (t * np.log(t / rho) + (1.0 - t) * np.log((1.0 - t) / (1.0 - rho))).mean())

    return (
        residual.astype(np.float32),
        op_probs.astype(np.float32),
        prior_scales.astype(np.float32),
        gate_probs.astype(np.float32),
        pc_recon,
        pc_entropy,
        kl,
    )


# revision 27
# speedup vs baseline: 1.0625x; 1.0625x over previous
"""Trainium2 Bass kernel for nn_CorticalSheet (MoE routing / predictive-coder columns).

Strategy: expert-parallel over the C=16 columns across 8 NeuronCores (2 experts
per core). Each core computes, for its 2 experts, the full 5-matmul chain
(enc D->K, 3x K->K, dec K->D) over all B*T=4096 rows in a transposed
[feature, rows] layout (so no on-chip transposes are needed), plus all the
heavy reductions: residual sums over T, squared-error partials, z sums over T,
and entropy partial sums (S1 = sum_k exp(h), S2 = sum_k h*exp(h)) via
ones-vector matmuls for the cross-partition reduction.

The cheap O(B*D) glue (gate layernorm/top-k/softmax, gated combine, op/prior
heads, KL) runs on host in numpy.

Two device-kernel variants:
- "bf16" (default): fully fused single pass per expert, all weights resident
  in SBUF, no DRAM round-trips; entropy chunk sums tree-reduced on DVE so the
  PE only runs one ones-matmul per quantity per block. ~1.65-1.9 ms HW time
  (chip P-state dependent), worst rel err ~1.2e-3.
- "f32r": 3-pass structure with rolling 32KB weight slots and fp32r (TF32-like)
  matmuls. ~2.19 ms, worst rel err ~8e-5. Select with KERNEL_VARIANT env.
"""

import sys

sys.path.insert(0, "/opt/trn_rl_repo")

import numpy as np

import concourse.bass as bass
import concourse.mybir as mybir
import concourse.tile as tile
from concourse.bass_utils import run_bass_kernel_spmd


def _ensure_axon_ntff_hook():
    """Register the axon NTFF profiling hook if the container's antenv stub
    lacks axon_hooks (boot skips registration silently in that case)."""
    import types

    try:
        import antenv.axon_hooks  # noqa: F401
        return
    except ImportError:
        pass
    try:
        from trn_agent_boot.trn_boot import _ntff_profile_via_ctypes
        hook = _ntff_profile_via_ctypes("/opt/axon/libaxon_pjrt.so")
    except Exception:
        hook = None
    mod = types.ModuleType("antenv.axon_hooks")
    state = {"hook": hook}
    mod.get_axon_ntff_profile_hook = lambda: state["hook"]
    mod.set_axon_ntff_profile_hook = lambda h: state.update(hook=h)
    sys.modules["antenv.axon_hooks"] = mod
    import antenv
    antenv.axon_hooks = mod


_ensure_axon_ntff_hook()

# ---- problem constants (hardcoded per contract) ----
B, T, D = 256, 16, 2048
C = 16
K = 1024
DEPTH = 4
TOPK = 4
TEMP = 1.0
SMAX = 10.0

P = 128
DO = D // P          # 16 d-chunks
KO = K // P          # 8 k-chunks
N = B * T            # 4096 rows
NCORES = 8
NE = C // NCORES     # 2 experts per core

NBA = 512            # rows per block, enc pass
NBB = 512            # rows per block, hidden pass
NBC = 512            # rows per block, dec pass

f32 = mybir.dt.float32
f32r = mybir.dt.float32r
bf16 = mybir.dt.bfloat16
AF = mybir.ActivationFunctionType
ALU = mybir.AluOpType
AXL = mybir.AxisListType
GELU_FN = AF.Gelu_apprx_tanh  # swapped to AF.Tanh for CoreSim (no Gelu LUT in sim)

_CACHE = {}
VARIANT = "bf16"  # default device-kernel variant ("f32r" fallback: ~2.19ms, rel err 8e-5)

LAST_RESULTS = None  # BassKernelResults of the most recent device run


def _legalize_multi_waits(nc):
    """Split instructions carrying >1 semaphore wait (or >1 update).

    The TPB EVENTS field holds exactly one wait and one update; this
    walrus_driver refuses to synthesize extra wait uops ("Too many sync wait
    commands"), so hoist the surplus waits into standalone EventSemaphore
    instructions on the same engine immediately before the instruction (and
    surplus updates into one immediately after).
    """
    n_ev = 0
    for func in nc.m.functions:
        for bb in func.blocks:
            insts = bb.instructions
            out = []
            changed = False
            for inst in insts:
                si = inst.sync_info
                waits = list(si.on_wait) if si is not None and si.on_wait else []
                updates = list(si.on_update) if si is not None and si.on_update else []
                if len(waits) > 1:
                    for w in waits[:-1]:
                        n_ev += 1
                        ev = mybir.InstEventSemaphore(
                            name=f"EVW-{n_ev}", ins=[], outs=[],
                            sync_info=mybir.SyncInfo(on_wait=[w], on_update=[]),
                        )
                        ev.engine = inst.engine
                        out.append(ev)
                    si.on_wait = [waits[-1]]
                    changed = True
                out.append(inst)
                if len(updates) > 1:
                    for u in updates[1:]:
                        n_ev += 1
                        ev = mybir.InstEventSemaphore(
                            name=f"EVU-{n_ev}", ins=[], outs=[],
                            sync_info=mybir.SyncInfo(on_wait=[], on_update=[u]),
                        )
                        ev.engine = inst.engine
                        out.append(ev)
                    si.on_update = [updates[0]]
                    changed = True
            if changed:
                bb.instructions = out
    return n_ev


def _build_bass():
    """Build the per-core Bass program (same program all 8 cores; data differs)."""
    nc = bass.Bass(trn_type="TRN2")

    xT = nc.dram_tensor("xT", [P, DO, N], f32r, kind="ExternalInput")
    w0 = [nc.dram_tensor(f"w0_{e}", [P, DO, K], f32r, kind="ExternalInput") for e in range(NE)]
    w1 = [nc.dram_tensor(f"w1_{e}", [P, KO, K], f32r, kind="ExternalInput") for e in range(NE)]
    w2 = [nc.dram_tensor(f"w2_{e}", [P, KO, K], f32r, kind="ExternalInput") for e in range(NE)]
    w3 = [nc.dram_tensor(f"w3_{e}", [P, KO, K], f32r, kind="ExternalInput") for e in range(NE)]
    wd = [nc.dram_tensor(f"wd_{e}", [P, KO, D], f32r, kind="ExternalInput") for e in range(NE)]
    b0 = [nc.dram_tensor(f"b0_{e}", [P, KO], f32, kind="ExternalInput") for e in range(NE)]
    b1 = [nc.dram_tensor(f"b1_{e}", [P, KO], f32, kind="ExternalInput") for e in range(NE)]
    b2 = [nc.dram_tensor(f"b2_{e}", [P, KO], f32, kind="ExternalInput") for e in range(NE)]
    b3 = [nc.dram_tensor(f"b3_{e}", [P, KO], f32, kind="ExternalInput") for e in range(NE)]
    bd = [nc.dram_tensor(f"bd_{e}", [P, DO], f32, kind="ExternalInput") for e in range(NE)]

    res_out = nc.dram_tensor("res_out", [NE, P, DO, B], f32, kind="ExternalOutput")
    z_out = nc.dram_tensor("z_out", [NE, P, KO, B], f32, kind="ExternalOutput")
    sq_out = nc.dram_tensor("sq_out", [NE, P, DO], f32, kind="ExternalOutput")
    ent_out = nc.dram_tensor("ent_out", [NE, 2, N], f32, kind="ExternalOutput")

    HDO = DO // 2  # 8
    HKO = KO // 2  # 4
    with tile.TileContext(nc) as tc:
        with (
            tc.tile_pool(name="dram", bufs=1, space="DRAM") as dpool,
            tc.tile_pool(name="const", bufs=1) as cpool,
            tc.tile_pool(name="wg", bufs=3) as wpool,
            tc.tile_pool(name="small", bufs=1) as smpool,
            tc.tile_pool(name="psg", bufs=6, space="PSUM") as psg,
            tc.tile_pool(name="ps1g", bufs=1, space="PSUM") as ps1g,
        ):
            ones_f = cpool.tile([P, 1], f32)
            nc.vector.memset(ones_f[:], 1.0)
            ones = ones_f[:].bitcast(f32r)
            # all biases, both experts, loaded up front (tiny)
            bias = {}
            for e in range(NE):
                for nm, src, w in (("b0", b0[e], KO), ("b1", b1[e], KO),
                                   ("b2", b2[e], KO), ("b3", b3[e], KO),
                                   ("bd", bd[e], DO)):
                    t = cpool.tile([P, w], f32, tag=f"{nm}_{e}")
                    nc.gpsimd.dma_start(t[:], src[:])
                    bias[(nm, e)] = t

            for e in range(NE):
                h0d = dpool.tile([P, KO, N], f32r, tag="h0")
                h3d = dpool.tile([P, KO, N], f32r, tag="h3")

                # ---------- pass A: enc (X -> h0) ----------
                # weights split by output column into two 32KB rolling slots
                w0a = wpool.tile([P, DO, K // 2], f32r, tag="w")
                nc.sync.dma_start(w0a[:], w0[e][:, :, :K // 2])
                w0b = wpool.tile([P, DO, K // 2], f32r, tag="w")
                nc.sync.dma_start(w0b[:], w0[e][:, :, K // 2:])
                b0t = bias[("b0", e)]
                with tc.tile_pool(name=f"aA{e}", bufs=3) as apool:
                    for blk in range(N // NBA):
                        sl = slice(blk * NBA, (blk + 1) * NBA)
                        x0 = apool.tile([P, HDO, NBA], f32r, tag="x")
                        nc.sync.dma_start(x0[:], xT[:, :HDO, sl])
                        x1 = apool.tile([P, HDO, NBA], f32r, tag="x")
                        nc.sync.dma_start(x1[:], xT[:, HDO:, sl])
                        h0t = apool.tile([P, KO, NBA], f32r, tag="h")
                        for ko in range(KO):
                            wt = w0a if ko < HKO else w0b
                            koo = ko % HKO
                            ps = psg.tile([P, NBA], f32, tag="mm")
                            for kd in range(DO):
                                xx = x0 if kd < HDO else x1
                                nc.tensor.matmul(
                                    ps[:],
                                    wt[:, kd, koo * P:(koo + 1) * P],
                                    xx[:, kd % HDO, :],
                                    start=(kd == 0),
                                    stop=(kd == DO - 1),
                                )
                            nc.scalar.activation(
                                h0t[:, ko, :], ps[:], GELU_FN,
                                bias=b0t[:, ko:ko + 1],
                            )
                        nc.sync.dma_start(h0d[:, :, sl], h0t[:])

                # ---------- pass B: hidden x3 (h0 -> h3) ----------
                wts = []
                for wi in (w1[e], w2[e], w3[e]):
                    wt = wpool.tile([P, KO, K], f32r, tag="w")
                    nc.sync.dma_start(wt[:], wi[:])
                    wts.append(wt)
                bts = [bias[("b1", e)], bias[("b2", e)], bias[("b3", e)]]
                with (
                    tc.tile_pool(name=f"aB{e}", bufs=2) as apool,
                    tc.tile_pool(name=f"sB{e}", bufs=1) as spool,
                ):
                    for blk in range(N // NBB):
                        sl = slice(blk * NBB, (blk + 1) * NBB)
                        hin = apool.tile([P, KO, NBB], f32r, tag="hin")
                        nc.sync.dma_start(hin[:], h0d[:, :, sl])
                        h1t = spool.tile([P, KO, NBB], f32r, tag="h1")
                        h2t = spool.tile([P, KO, NBB], f32r, tag="h2")
                        h3t = apool.tile([P, KO, NBB], f32r, tag="h3o")
                        for src, dst, li in ((hin, h1t, 0), (h1t, h2t, 1), (h2t, h3t, 2)):
                            for ko in range(KO):
                                ps = psg.tile([P, NBB], f32, tag="mm")
                                for kd in range(KO):
                                    nc.tensor.matmul(
                                        ps[:],
                                        wts[li][:, kd, ko * P:(ko + 1) * P],
                                        src[:, kd, :],
                                        start=(kd == 0),
                                        stop=(kd == KO - 1),
                                    )
                                nc.scalar.activation(
                                    dst[:, ko, :], ps[:], GELU_FN,
                                    bias=bts[li][:, ko:ko + 1],
                                )
                        nc.sync.dma_start(h3d[:, :, sl], h3t[:])

                # ---------- pass C: dec + stats ----------
                wda = wpool.tile([P, KO, D // 2], f32r, tag="w")
                nc.sync.dma_start(wda[:], wd[e][:, :, :D // 2])
                wdb = wpool.tile([P, KO, D // 2], f32r, tag="w")
                nc.sync.dma_start(wdb[:], wd[e][:, :, D // 2:])
                bdt = bias[("bd", e)]

                sq_sb = smpool.tile([P, DO], f32, tag=f"sq{e}")
                nc.vector.memset(sq_sb[:], 0.0)

                with (
                    tc.tile_pool(name=f"aC{e}", bufs=2) as apool,
                    tc.tile_pool(name=f"eC{e}", bufs=2) as epool,
                    tc.tile_pool(name=f"e1C{e}", bufs=1) as e1pool,
                ):
                    bblk = NBC // T  # batch entries per block
                    for blk in range(N // NBC):
                        sl = slice(blk * NBC, (blk + 1) * NBC)
                        bsl = slice(blk * bblk, (blk + 1) * bblk)
                        h3t = apool.tile([P, KO, NBC], f32r, tag="h3in")
                        nc.sync.dma_start(h3t[:], h3d[:, :, sl])
                        xc0 = apool.tile([P, HDO, NBC], f32r, tag="xc")
                        nc.sync.dma_start(xc0[:], xT[:, :HDO, sl])
                        xc1 = apool.tile([P, HDO, NBC], f32r, tag="xc")
                        nc.sync.dma_start(xc1[:], xT[:, HDO:, sl])

                        # entropy partials + z sums
                        ps1 = ps1g.tile([1, NBC], f32, tag="s1")
                        ps2 = ps1g.tile([1, NBC], f32, tag="s2")
                        z_blk = epool.tile([P, KO, bblk], f32, tag="zblk")
                        for kc in range(KO):
                            et = epool.tile([P, NBC], f32r, tag="e")
                            nc.scalar.activation(et[:], h3t[:, kc, :], AF.Exp)
                            het = epool.tile([P, NBC], f32r, tag="he")
                            nc.vector.tensor_tensor(het[:], et[:], h3t[:, kc, :], ALU.mult)
                            nc.tensor.matmul(ps1[:], ones[:, :1], et[:],
                                             start=(kc == 0), stop=(kc == KO - 1))
                            nc.tensor.matmul(ps2[:], ones[:, :1], het[:],
                                             start=(kc == 0), stop=(kc == KO - 1))
                            nc.vector.tensor_reduce(
                                z_blk[:, kc, :],
                                h3t[:, kc, :].rearrange("p (b t) -> p b t", t=T),
                                axis=AXL.X, op=ALU.add,
                            )
                        nc.sync.dma_start(z_out[e][:, :, bsl], z_blk[:])
                        s1t = e1pool.tile([1, NBC], f32, tag="s1sb")
                        nc.vector.tensor_copy(s1t[:], ps1[:])
                        nc.sync.dma_start(ent_out[e, 0, sl], s1t[:])
                        s2t = e1pool.tile([1, NBC], f32, tag="s2sb")
                        nc.vector.tensor_copy(s2t[:], ps2[:])
                        nc.sync.dma_start(ent_out[e, 1, sl], s2t[:])

                        # dec matmuls + error stats
                        res_blk = epool.tile([P, DO, bblk], f32, tag="resblk")
                        for dc in range(DO):
                            wt = wda if dc < HDO else wdb
                            dco = dc % HDO
                            ps = psg.tile([P, NBC], f32, tag="mm")
                            for kd in range(KO):
                                nc.tensor.matmul(
                                    ps[:],
                                    wt[:, kd, dco * P:(dco + 1) * P],
                                    h3t[:, kd, :],
                                    start=(kd == 0),
                                    stop=(kd == KO - 1),
                                )
                            err0 = epool.tile([P, NBC], f32, tag="err0")
                            nc.scalar.activation(err0[:], ps[:], AF.Identity,
                                                 bias=bdt[:, dc:dc + 1])
                            err = epool.tile([P, NBC], f32, tag="err")
                            xx = xc0 if dc < HDO else xc1
                            nc.vector.tensor_tensor(err[:], err0[:], xx[:, dc % HDO, :], ALU.subtract)
                            scr = e1pool.tile([P, NBC], f32, tag="scr")
                            sqp = e1pool.tile([P, 1], f32, tag="sqp")
                            nc.vector.tensor_tensor(scr[:], err[:], err[:], ALU.mult)
                            nc.vector.tensor_reduce(
                                sqp[:], scr[:], axis=AXL.X, op=ALU.add)
                            nc.vector.tensor_tensor(
                                sq_sb[:, dc:dc + 1], sq_sb[:, dc:dc + 1], sqp[:], ALU.add)
                            nc.vector.tensor_reduce(
                                res_blk[:, dc, :],
                                err.rearrange("p (b t) -> p b t", t=T),
                                axis=AXL.X, op=ALU.add,
                            )
                        nc.sync.dma_start(res_out[e][:, :, bsl], res_blk[:])

                nc.sync.dma_start(sq_out[e], sq_sb[:])

    _legalize_multi_waits(nc)
    return nc


def _build_bass_bf16():
    """Fully-fused bf16 variant: all weights resident per expert, the whole
    enc->h1->h2->h3->dec chain runs per row-block with no DRAM round-trips."""
    nc = bass.Bass(trn_type="TRN2")

    xT = nc.dram_tensor("xT", [P, DO, N], bf16, kind="ExternalInput")
    w0 = [nc.dram_tensor(f"w0_{e}", [P, DO, K], bf16, kind="ExternalInput") for e in range(NE)]
    w1 = [nc.dram_tensor(f"w1_{e}", [P, KO, K], bf16, kind="ExternalInput") for e in range(NE)]
    w2 = [nc.dram_tensor(f"w2_{e}", [P, KO, K], bf16, kind="ExternalInput") for e in range(NE)]
    w3 = [nc.dram_tensor(f"w3_{e}", [P, KO, K], bf16, kind="ExternalInput") for e in range(NE)]
    wd = [nc.dram_tensor(f"wd_{e}", [P, KO, D], bf16, kind="ExternalInput") for e in range(NE)]
    b0 = [nc.dram_tensor(f"b0_{e}", [P, KO], f32, kind="ExternalInput") for e in range(NE)]
    b1 = [nc.dram_tensor(f"b1_{e}", [P, KO], f32, kind="ExternalInput") for e in range(NE)]
    b2 = [nc.dram_tensor(f"b2_{e}", [P, KO], f32, kind="ExternalInput") for e in range(NE)]
    b3 = [nc.dram_tensor(f"b3_{e}", [P, KO], f32, kind="ExternalInput") for e in range(NE)]
    bd = [nc.dram_tensor(f"bd_{e}", [P, DO], f32, kind="ExternalInput") for e in range(NE)]

    res_out = nc.dram_tensor("res_out", [NE, P, DO, B], f32, kind="ExternalOutput")
    z_out = nc.dram_tensor("z_out", [NE, P, KO, B], f32, kind="ExternalOutput")
    sq_out = nc.dram_tensor("sq_out", [NE, P, DO], f32, kind="ExternalOutput")
    ent_out = nc.dram_tensor("ent_out", [NE, 2, N], f32, kind="ExternalOutput")

    NB = 512
    bblk = NB // T
    with tile.TileContext(nc) as tc:
        with (
            tc.tile_pool(name="const", bufs=1) as cpool,
            tc.tile_pool(name="wg", bufs=1) as wpool,
            tc.tile_pool(name="ap", bufs=2) as apool,
            tc.tile_pool(name="hp", bufs=2) as hpool,
            tc.tile_pool(name="ep", bufs=2) as epool,
            tc.tile_pool(name="e1", bufs=1) as e1pool,
            tc.tile_pool(name="small", bufs=1) as smpool,
            tc.tile_pool(name="psg", bufs=6, space="PSUM") as psg,
            tc.tile_pool(name="ps1g", bufs=1, space="PSUM") as ps1g,
        ):
            ones_f = cpool.tile([P, 1], f32)
            nc.vector.memset(ones_f[:], 1.0)
            ones = cpool.tile([P, 1], bf16)
            nc.vector.tensor_copy(ones[:], ones_f[:])
            bias = {}
            for e in range(NE):
                for nm, src, w in (("b0", b0[e], KO), ("b1", b1[e], KO),
                                   ("b2", b2[e], KO), ("b3", b3[e], KO),
                                   ("bd", bd[e], DO)):
                    t = cpool.tile([P, w], f32, tag=f"{nm}_{e}")
                    nc.gpsimd.dma_start(t[:], src[:])
                    bias[(nm, e)] = t

            for e in range(NE):
                w0t = wpool.tile([P, DO, K], bf16, tag="w0")
                nc.sync.dma_start(w0t[:], w0[e][:])
                # first x block right behind w0 so the first matmuls start early
                xt0 = apool.tile([P, DO, NB], bf16, tag="x")
                nc.sync.dma_start(xt0[:], xT[:, :, 0:NB])
                wts = []
                for li, wi in enumerate((w1[e], w2[e], w3[e])):
                    wt = wpool.tile([P, KO, K], bf16, tag=f"w{li + 1}")
                    nc.sync.dma_start(wt[:], wi[:])
                    wts.append(wt)
                wdt = wpool.tile([P, KO, D], bf16, tag="wd")
                nc.sync.dma_start(wdt[:], wd[e][:])

                sq_sb = smpool.tile([P, DO], f32, tag=f"sq{e}")
                nc.vector.memset(sq_sb[:], 0.0)

                for blk in range(N // NB):
                    sl = slice(blk * NB, (blk + 1) * NB)
                    bsl = slice(blk * bblk, (blk + 1) * bblk)
                    if blk == 0:
                        xt = xt0
                    else:
                        xt = apool.tile([P, DO, NB], bf16, tag="x")
                        nc.sync.dma_start(xt[:], xT[:, :, sl])

                    # enc
                    h0t = hpool.tile([P, KO, NB], bf16, tag="ha")
                    for ko in range(KO):
                        ps = psg.tile([P, NB], f32, tag="mm")
                        for kd in range(DO):
                            nc.tensor.matmul(
                                ps[:], w0t[:, kd, ko * P:(ko + 1) * P], xt[:, kd, :],
                                start=(kd == 0), stop=(kd == DO - 1))
                        nc.scalar.activation(h0t[:, ko, :], ps[:], GELU_FN,
                                             bias=bias[("b0", e)][:, ko:ko + 1])
                    # hidden x3 (ping-pong ha/hb)
                    hcur = h0t
                    for li in range(3):
                        hnxt = hpool.tile([P, KO, NB], bf16,
                                          tag="hb" if li % 2 == 0 else "ha")
                        for ko in range(KO):
                            ps = psg.tile([P, NB], f32, tag="mm")
                            for kd in range(KO):
                                nc.tensor.matmul(
                                    ps[:], wts[li][:, kd, ko * P:(ko + 1) * P], hcur[:, kd, :],
                                    start=(kd == 0), stop=(kd == KO - 1))
                            nc.scalar.activation(hnxt[:, ko, :], ps[:], GELU_FN,
                                                 bias=bias[(f"b{li + 1}", e)][:, ko:ko + 1])
                        hcur = hnxt
                    h3t = hcur

                    # entropy partials + z sums
                    ps1 = ps1g.tile([1, NB], f32, tag="s1")
                    ps2 = ps1g.tile([1, NB], f32, tag="s2")
                    z_blk = epool.tile([P, KO, bblk], f32, tag="zblk")
                    esum = epool.tile([P, NB], bf16, tag="esum")
                    hesum = epool.tile([P, NB], bf16, tag="hesum")
                    for kc in range(KO):
                        if kc == 0:
                            nc.scalar.activation(esum[:], h3t[:, kc, :], AF.Exp)
                            nc.vector.tensor_tensor(hesum[:], esum[:], h3t[:, kc, :], ALU.mult)
                        else:
                            et = epool.tile([P, NB], bf16, tag="e")
                            nc.scalar.activation(et[:], h3t[:, kc, :], AF.Exp)
                            het = epool.tile([P, NB], bf16, tag="he")
                            nc.vector.tensor_tensor(het[:], et[:], h3t[:, kc, :], ALU.mult)
                            nc.vector.tensor_tensor(esum[:], esum[:], et[:], ALU.add)
                            nc.vector.tensor_tensor(hesum[:], hesum[:], het[:], ALU.add)
                        nc.vector.tensor_reduce(
                            z_blk[:, kc, :],
                            h3t[:, kc, :].rearrange("p (b t) -> p b t", t=T),
                            axis=AXL.X, op=ALU.add)
                    nc.tensor.matmul(ps1[:], ones[:, :1], esum[:], start=True, stop=True)
                    nc.tensor.matmul(ps2[:], ones[:, :1], hesum[:], start=True, stop=True)
                    nc.sync.dma_start(z_out[e][:, :, bsl], z_blk[:])
                    s1t = e1pool.tile([1, NB], f32, tag="s1sb")
                    nc.vector.tensor_copy(s1t[:], ps1[:])
                    nc.sync.dma_start(ent_out[e, 0, sl], s1t[:])
                    s2t = e1pool.tile([1, NB], f32, tag="s2sb")
                    nc.vector.tensor_copy(s2t[:], ps2[:])
                    nc.sync.dma_start(ent_out[e, 1, sl], s2t[:])

                    # dec + error stats
                    res_blk = epool.tile([P, DO, bblk], f32, tag="resblk")
                    for dc in range(DO):
                        ps = psg.tile([P, NB], f32, tag="mm")
                        for kd in range(KO):
                            nc.tensor.matmul(
                                ps[:], wdt[:, kd, dc * P:(dc + 1) * P], h3t[:, kd, :],
                                start=(kd == 0), stop=(kd == KO - 1))
                        err0 = epool.tile([P, NB], f32, tag="err0")
                        nc.scalar.activation(err0[:], ps[:], AF.Identity,
                                             bias=bias[("bd", e)][:, dc:dc + 1])
                        xf = e1pool.tile([P, NB], f32, tag="xf")
                        nc.scalar.copy(xf[:], xt[:, dc, :])
                        err = epool.tile([P, NB], f32, tag="err")
                        nc.vector.tensor_tensor(err[:], err0[:], xf[:], ALU.subtract)
                        scr = e1pool.tile([P, NB], f32, tag="scr")
                        sqp = e1pool.tile([P, 1], f32, tag="sqp")
                        nc.vector.tensor_tensor(scr[:], err[:], err[:], ALU.mult)
                        nc.vector.tensor_reduce(sqp[:], scr[:], axis=AXL.X, op=ALU.add)
                        nc.vector.tensor_tensor(
                            sq_sb[:, dc:dc + 1], sq_sb[:, dc:dc + 1], sqp[:], ALU.add)
                        nc.vector.tensor_reduce(
                            res_blk[:, dc, :],
                            err.rearrange("p (b t) -> p b t", t=T),
                            axis=AXL.X, op=ALU.add)
                    nc.sync.dma_start(res_out[e][:, :, bsl], res_blk[:])

                nc.sync.dma_start(sq_out[e], sq_sb[:])

    _legalize_multi_waits(nc)
    return nc


def _prep_inputs(inputs, wdtype=np.float32):
    """Host-side shard/layout prep -> list of 8 per-core input dicts."""
    tokens = np.ascontiguousarray(inputs["tokens"], dtype=np.float32)
    X = tokens.reshape(N, D)
    # xT[p, o, n] = X[n, o*128 + p]
    xT = np.ascontiguousarray(X.T.reshape(DO, P, N).transpose(1, 0, 2).astype(wdtype))

    ew0 = np.asarray(inputs["ew0"], dtype=np.float32)
    ew = np.asarray(inputs["ew"], dtype=np.float32)
    dw = np.asarray(inputs["dw"], dtype=np.float32)
    eb0 = np.asarray(inputs["eb0"], dtype=np.float32)
    eb = np.asarray(inputs["eb"], dtype=np.float32)
    db = np.asarray(inputs["db"], dtype=np.float32)

    in_maps = []
    for core in range(NCORES):
        m = {"xT": xT}
        for e in range(NE):
            c = core * NE + e
            m[f"w0_{e}"] = np.ascontiguousarray(
                ew0[c].reshape(DO, P, K).transpose(1, 0, 2).astype(wdtype))
            for i in range(DEPTH - 1):
                m[f"w{i + 1}_{e}"] = np.ascontiguousarray(
                    ew[i, c].reshape(KO, P, K).transpose(1, 0, 2).astype(wdtype))
                m[f"b{i + 1}_{e}"] = np.ascontiguousarray(
                    eb[i, c].reshape(KO, P).T)
            m[f"wd_{e}"] = np.ascontiguousarray(
                dw[c].reshape(KO, P, D).transpose(1, 0, 2).astype(wdtype))
            m[f"b0_{e}"] = np.ascontiguousarray(eb0[c].reshape(KO, P).T)
            m[f"bd_{e}"] = np.ascontiguousarray(db[c].reshape(DO, P).T)
        in_maps.append(m)
    return in_maps


def _gelu_tanh(x):
    return 0.5 * x * (1.0 + np.tanh(np.sqrt(2.0 / np.pi) * (x + 0.044715 * x ** 3)))


def _ln_np(x, s, b):
    m = x.mean(-1, keepdims=True)
    v = x.var(-1, keepdims=True)
    return (x - m) / np.sqrt(v + 1e-5) * s + b


def _softmax_np(x):
    x = x - x.max(-1, keepdims=True)
    ex = np.exp(x)
    return ex / ex.sum(-1, keepdims=True)


def kernel(**inputs):
    global LAST_RESULTS
    import os
    variant = os.environ.get("KERNEL_VARIANT", VARIANT)
    if variant not in _CACHE:
        _CACHE[variant] = _build_bass() if variant == "f32r" else _build_bass_bf16()
    nc = _CACHE[variant]

    import ml_dtypes
    wdtype = np.float32 if variant == "f32r" else ml_dtypes.bfloat16
    in_maps = _prep_inputs(inputs, wdtype)
    import os
    trace = bool(int(os.environ.get("KERNEL_TRACE", "0")))
    results = run_bass_kernel_spmd(
        nc, in_maps, core_ids=list(range(NCORES)), trace=trace,
    )
    LAST_RESULTS = results
    per_core = results.results

    # ---- reassemble device outputs ----
    residuals = np.empty((C, B, D), np.float64)
    zc = np.empty((C, B, K), np.float64)
    sq_total = 0.0
    ent_sum = 0.0
    for core in range(NCORES):
        r = per_core[core]
        for e in range(NE):
            c = core * NE + e
            # res_out[e]: [P, DO, B] -> [B, DO*P] with d = o*128+p
            residuals[c] = r["res_out"][e].transpose(2, 1, 0).reshape(B, D) / T
            zc[c] = r["z_out"][e].transpose(2, 1, 0).reshape(B, K) / T
            sq_total += float(r["sq_out"][e].astype(np.float64).sum())
            s1 = r["ent_out"][e, 0].astype(np.float64)
            s2 = r["ent_out"][e, 1].astype(np.float64)
            ent_sum += float((np.log(s1) - s2 / s1).sum())

    pc_recon = np.float32(sq_total / (C * B * T * D))
    pc_entropy = np.float32(ent_sum / (C * N))

    # ---- gate (host) ----
    tokens = np.asarray(inputs["tokens"], dtype=np.float32)
    gln_s = np.asarray(inputs["gln_s"], np.float32)
    gln_b = np.asarray(inputs["gln_b"], np.float32)
    gw = np.asarray(inputs["gw"], np.float32)
    gb = np.asarray(inputs["gb"], np.float32)

    x = tokens.mean(axis=1).astype(np.float64)
    logits = (_ln_np(x, gln_s, gln_b) @ gw + gb) / max(1e-4, TEMP)
    top_idx = np.argsort(-logits, axis=-1)[:, :TOPK]
    mask = np.zeros_like(logits)
    np.put_along_axis(mask, top_idx, 1.0, axis=-1)
    gp = _softmax_np(logits) * mask
    gate_probs = gp / np.maximum(gp.sum(-1, keepdims=True), 1e-6)

    # ---- gated combine ----
    residual = np.einsum("bc,cbd->bd", gate_probs, residuals)
    latent = np.einsum("bc,cbk->bk", gate_probs, zc)

    # ---- heads ----
    oln_s = np.asarray(inputs["oln_s"], np.float32)
    oln_b = np.asarray(inputs["oln_b"], np.float32)
    ow1 = np.asarray(inputs["ow1"], np.float32)
    ob1 = np.asarray(inputs["ob1"], np.float32)
    ow2 = np.asarray(inputs["ow2"], np.float32)
    ob2 = np.asarray(inputs["ob2"], np.float32)
    pln_s = np.asarray(inputs["pln_s"], np.float32)
    pln_b = np.asarray(inputs["pln_b"], np.float32)
    pw1 = np.asarray(inputs["pw1"], np.float32)
    pb1 = np.asarray(inputs["pb1"], np.float32)
    pw2 = np.asarray(inputs["pw2"], np.float32)
    pb2 = np.asarray(inputs["pb2"], np.float32)

    op_logits = _gelu_tanh(_ln_np(latent, oln_s, oln_b) @ ow1 + ob1) @ ow2 + ob2
    op_probs = _softmax_np(op_logits)
    prior_raw = 1.0 / (1.0 + np.exp(-(_gelu_tanh(_ln_np(latent, pln_s, pln_b) @ pw1 + pb1) @ pw2 + pb2)))
    smin = 1.0 / SMAX
    prior_scales = smin + (SMAX - smin) * prior_raw

    # ---- kl sparsity ----
    t = 0.05
    rho = np.clip((1.0 / (1.0 + np.exp(-latent))).mean(axis=0), 1e-6, 1.0 - 1e-6)
    kl = np.float32(
        (t * np.log(t / rho) + (1.0 - t) * np.log((1.0 - t) / (1.0 - rho))).mean())

    return (
        residual.astype(np.float32),
        op_probs.astype(np.float32),
        prior_scales.astype(np.float32),
        gate_probs.astype(np.float32),
        pc_recon,
        pc_entropy,
        kl,
    )
